# revision 17
# baseline (speedup 1.0000x reference)
"""Trainium2 Bass kernel for the N^3 triplet descriptor (gnn_message_passing).

Strategy: the reference's O(N^3) angular sum factorizes exactly via the
Legendre addition theorem into O(N^2) per-pair vector moments:

  P0 term: (sum_j w_j)^2
  P1 term: |sum_j w_j u_j|^2                  (u = unit displacement)
  P2 term: 1.5*|sum_j w_j u_j u_j^T|_F^2 - 0.5*(sum_j w_j)^2

with w_j = fc(r_ij) * r_ij^n.  Each device accumulates 36 pair moments per
central atom (9 radial powers, 9 S1 components, 9+9 symmetric S2
components); the tiny nonlinear combine runs on host after gathering.

Sharding: 8 cores = 2 i-blocks (96 rows on partitions) x 4 j-chunks (48
neighbors on the free axis). Cross-j-chunk partials are summed on host.

Implementation: raw Bass (no Tile framework) with per-engine semaphore
chains. Work is split DVE / GpSimd / ACT to shorten the DVE critical
path; the single ACT table (abs_reciprocal_sqrt_and_small) provides
1/r = 1/sqrt(r^2+eps) in one op, r = r2 * rinv on DVE; fc is a degree-6
polynomial in r^2 with an exact (r^2 < RC^2) cutoff mask.
"""

import numpy as np

import concourse.bass as bass
import concourse.bacc as bacc
from concourse import mybir
from concourse.bass_utils import run_bass_kernel_spmd

F32 = mybir.dt.float32
ALU = mybir.AluOpType
ACT = mybir.ActivationFunctionType

N = 192
NI = 96          # i rows per core (partition dim)
NJ = 48          # j neighbors per core (free dim)
NIB = 2          # i blocks
NJC = 4          # j chunks
BOX_L = 20.0
RC = 5.0

# fc(w) = 0.5*(1+cos(pi*sqrt(w)/RC)) as degree-6 poly in w = r^2, w in [0, RC^2]
# (chebyshev fit, max abs err 1.3e-8)
_FC_W = np.linspace(0, RC * RC, 20001)
_FC_Y = 0.5 * (1 + np.cos(np.pi * np.sqrt(_FC_W) / RC))
_FC_C = (
    np.polynomial.chebyshev.Chebyshev.fit(_FC_W, _FC_Y, 6, domain=[0, RC * RC])
    .convert(kind=np.polynomial.Polynomial)
    .coef.astype(np.float64)
)

_cached = {}


def _v(ap, off, dims):
    """Custom free-dim view of an SBUF tile AP: keep partition dim, replace
    free dims, shift offset by `off` elements."""
    return bass.AP(ap.tensor, ap.offset + off, [list(ap.ap[0])] + [list(d) for d in dims])


def build_nc():
    nc = bacc.Bacc(
        "TRN2",
        target_bir_lowering=False,
        debug=False,
        enable_asserts=True,
        num_devices=NIB * NJC,
    )
    rji_d = nc.dram_tensor("rji", [NI, 160], F32, kind="ExternalInput").ap()
    out_d = nc.dram_tensor("out", [NI, 36], F32, kind="ExternalOutput").ap()

    rji = nc.alloc_sbuf_tensor("rji_s", [NI, 160], F32).ap()
    dxr = nc.alloc_sbuf_tensor("dxr", [NI, 144], F32).ap()
    dx = nc.alloc_sbuf_tensor("dx", [NI, 144], F32).ap()
    sq_t = nc.alloc_sbuf_tensor("sq_t", [NI, 144], F32).ap()
    r2 = nc.alloc_sbuf_tensor("r2", [NI, NJ], F32).ap()
    rv = nc.alloc_sbuf_tensor("rv", [NI, 144], F32).ap()   # [rinv | ones | r]
    m25 = nc.alloc_sbuf_tensor("m25", [NI, NJ], F32).ap()
    yh = nc.alloc_sbuf_tensor("yh", [NI, NJ], F32).ap()
    yh144 = nc.alloc_sbuf_tensor("yh144", [NI, 144], F32).ap()
    r4 = nc.alloc_sbuf_tensor("r4", [NI, NJ], F32).ap()
    poff = nc.alloc_sbuf_tensor("poff", [NI, 144], F32).ap()
    fcp = nc.alloc_sbuf_tensor("fcp", [NI, 9 * NJ], F32).ap()
    w1 = nc.alloc_sbuf_tensor("w1", [NI, 144], F32).ap()
    w2 = nc.alloc_sbuf_tensor("w2", [NI, 144], F32).ap()
    big3 = nc.alloc_sbuf_tensor("big3", [NI, 1296], F32).ap()  # T | bigd | bigo
    sg = nc.alloc_sbuf_tensor("sg", [NI, 36], F32).ap()
    scr = nc.alloc_sbuf_tensor("scr", [1, 8], F32).ap()

    dsem = nc.alloc_semaphore("dsem")
    vq = nc.alloc_semaphore("vq")      # DVE instruction counter
    sqm = nc.alloc_semaphore("sqm")    # ACT instruction counter
    gq = nc.alloc_semaphore("gq")      # GpSimd instruction counter

    rj3 = rji[:, 0:144].rearrange("p (d j) -> p d j", d=3)
    ri3 = rji[:, 144:147].unsqueeze(-1).broadcast_to((NI, 3, NJ))
    dxr3 = dxr.rearrange("p (d j) -> p d j", d=3)
    dx3 = dx.rearrange("p (d j) -> p d j", d=3)
    rinv = rv[:, 0:NJ]
    r = rv[:, 2 * NJ:3 * NJ]
    rinv3 = rinv.unsqueeze(1).broadcast_to((NI, 3, NJ))
    fc = fcp[:, 0:NJ]

    c = [float(x) for x in _FC_C]

    # cross-engine wait points (per-engine instruction-counter values)
    VQ_DX = 5      # dx ready
    VQ_R2 = 8      # r2 (+eps) ready
    VQ_W2 = 19     # w2 ready
    VQ_QR = 24     # radial moments in sg
    VQ_ALL = 27    # sg complete
    SQ_RINV = 2    # rinv ready
    GQ_ONES = 1    # rv ones block set
    GQ_GEO = 4     # poff + r4 ready
    GQ_BIGD = 5
    GQ_BIGO = 6

    with nc.Block() as block:

        @block.sync
        def _(sync):
            sync.dma_start(rji, rji_d).then_inc(dsem, 16)
            sync.wait_ge(vq, VQ_QR)
            sync.dma_start(out_d[:, 0:9], sg[:, 0:9]).then_inc(dsem, 16)
            sync.wait_ge(vq, VQ_ALL)
            sync.dma_start(out_d[:, 9:36], sg[:, 9:36]).then_inc(dsem, 16)
            sync.wait_ge(dsem, 48)

        @block.scalar
        def _(scalar):
            sn = [0]

            def S(inst):
                # same-engine ordering chain (TRN2 engines pipeline;
                # RAW hazards need explicit sems — free at runtime)
                if sn[0] > 0:
                    inst._wait_ge(sqm, sn[0])
                inst.then_inc(sqm, 1)
                sn[0] += 1
                return inst

            # dummy activation on a const tile: pulls the single ACT table
            # load (abs_reciprocal_sqrt_and_small) to t=0, overlapped with
            # the input DMA + DVE distance math
            S(scalar.activation(
                scr[0:1, 0:1], nc.const_aps.tensor(1.0, (1, 1)),
                ACT.Abs_reciprocal_sqrt))
            scalar.wait_ge(vq, VQ_R2)
            # rinv = 1/sqrt(r2 + 1e-12); r recovered on DVE as r2 * rinv
            S(scalar.activation(rinv, r2, ACT.Abs_reciprocal_sqrt))
            assert sn[0] == SQ_RINV

        @block.gpsimd
        def _(gpsimd):
            gn = [0]

            def G(inst):
                if gn[0] > 0:
                    inst._wait_ge(gq, gn[0])
                inst.then_inc(gq, 1)
                gn[0] += 1
                return inst

            # middle block of rv is all-ones (for the one-shot w1 product)
            G(gpsimd.memset(rv[:, NJ:2 * NJ], 1.0))
            # off-critical-path geometry, freeing the DVE
            gpsimd.wait_ge(vq, VQ_DX)
            G(gpsimd.tensor_tensor(
                poff[:, 0:96], dx[:, 0:96], dx[:, 48:144], op=ALU.mult))
            G(gpsimd.tensor_tensor(
                poff[:, 96:144], dx[:, 0:NJ], dx[:, 96:144], op=ALU.mult))
            gpsimd.wait_ge(vq, VQ_R2)
            G(gpsimd.tensor_tensor(r4, r2, r2, op=ALU.mult))
            assert gn[0] == GQ_GEO
            # S2 products while the DVE runs powers/radial/T
            gpsimd.wait_ge(vq, VQ_W2)
            G(gpsimd.tensor_tensor(
                _v(big3, 432, [[144, 3], [NJ, 3], [1, NJ]]),
                w2.rearrange("p (n j) -> p n j", n=3).unsqueeze(2).broadcast_to((NI, 3, 3, NJ)),
                sq_t.rearrange("p (d j) -> p d j", d=3).unsqueeze(1).broadcast_to((NI, 3, 3, NJ)),
                op=ALU.mult))
            assert gn[0] == GQ_BIGD
            G(gpsimd.tensor_tensor(
                _v(big3, 864, [[144, 3], [NJ, 3], [1, NJ]]),
                w2.rearrange("p (n j) -> p n j", n=3).unsqueeze(2).broadcast_to((NI, 3, 3, NJ)),
                poff.rearrange("p (m j) -> p m j", m=3).unsqueeze(1).broadcast_to((NI, 3, 3, NJ)),
                op=ALU.mult))
            assert gn[0] == GQ_BIGO

        @block.vector
        def _(vector):
            vn = [0]

            def V(inst):
                if vn[0] > 0:
                    inst._wait_ge(vq, vn[0])
                inst.then_inc(vq, 1)
                vn[0] += 1
                return inst

            vector.wait_ge(dsem, 16)
            V(vector.tensor_tensor(dxr3, rj3, ri3, op=ALU.subtract))
            # minimum image (box = BOX_L * I): dx -= L*(dxr>L/2); dx += L*(dxr<-L/2)
            V(vector.tensor_scalar(
                yh144, dxr, BOX_L / 2, BOX_L, op0=ALU.is_gt, op1=ALU.mult))
            V(vector.tensor_tensor(dx, dxr, yh144, op=ALU.subtract))
            V(vector.tensor_scalar(
                yh144, dxr, -BOX_L / 2, BOX_L, op0=ALU.is_lt, op1=ALU.mult))
            V(vector.tensor_tensor(dx, dx, yh144, op=ALU.add))
            assert vn[0] == VQ_DX
            V(vector.tensor_tensor(sq_t, dx, dx, op=ALU.mult))
            V(vector.reduce_sum(
                r2, sq_t.rearrange("p (d j) -> p j d", d=3),
                axis=mybir.AxisListType.X,
            ))
            # eps keeps 1/sqrt finite at the self pair (u_ii ends up 0)
            V(vector.tensor_scalar(r2, r2, 1e-12, None, op0=ALU.add))
            assert vn[0] == VQ_R2
            # fc = poly6(r2) * (r2 < RC^2), Horner on DVE
            V(vector.tensor_scalar(m25, r2, RC * RC, None, op0=ALU.is_lt))
            V(vector.tensor_scalar(yh, r2, c[6], None, op0=ALU.mult))
            for k in (5, 4, 3, 2, 1):
                V(vector.scalar_tensor_tensor(
                    yh, yh, c[k], r2, op0=ALU.add, op1=ALU.mult))
            V(vector.scalar_tensor_tensor(
                fc, yh, c[0], m25, op0=ALU.add, op1=ALU.mult))
            assert vn[0] == 16
            # r = r2 * rinv (= sqrt(r2+eps)) into rv's third block
            vector.wait_ge(sqm, SQ_RINV)
            V(vector.tensor_tensor(r, r2, rinv, op=ALU.mult))
            # weights in one shot: w1_n = fc * [rinv|1|r], w2_n = w1_n * rinv
            vector.wait_ge(gq, GQ_ONES)
            V(vector.tensor_tensor(
                w1, _v(fcp, 0, [[0, 3], [1, NJ]]), rv, op=ALU.mult))
            V(vector.tensor_tensor(
                w2.rearrange("p (n j) -> p n j", n=3),
                w1.rearrange("p (n j) -> p n j", n=3),
                rinv3, op=ALU.mult))
            assert vn[0] == VQ_W2
            # fcp[k] = fc * r^k: evens via r2/r4, odds in one strided mult
            vector.wait_ge(gq, GQ_GEO)
            V(vector.tensor_tensor(
                fcp[:, 2 * NJ:3 * NJ], fc, r2, op=ALU.mult))
            V(vector.tensor_tensor(
                _v(fcp, 4 * NJ, [[2 * NJ, 2], [1, NJ]]),
                _v(fcp, 0, [[2 * NJ, 2], [1, NJ]]),
                _v(r4, 0, [[0, 2], [1, NJ]]),
                op=ALU.mult,
            ))
            V(vector.tensor_tensor(
                fcp[:, 8 * NJ:9 * NJ], fcp[:, 4 * NJ:5 * NJ], r4, op=ALU.mult))
            V(vector.tensor_tensor(
                _v(fcp, NJ, [[2 * NJ, 4], [1, NJ]]),
                _v(fcp, 0, [[2 * NJ, 4], [1, NJ]]),
                _v(r, 0, [[0, 4], [1, NJ]]),
                op=ALU.mult,
            ))
            V(vector.reduce_sum(
                sg[:, 0:9], fcp.rearrange("p (k j) -> p k j", k=9),
                axis=mybir.AxisListType.X,
            ))
            assert vn[0] == VQ_QR
            # S1 products: T[n,d] = w1_n * dx_d into big3[0:432]
            V(vector.tensor_tensor(
                _v(big3, 0, [[144, 3], [NJ, 3], [1, NJ]]),
                w1.rearrange("p (n j) -> p n j", n=3).unsqueeze(2).broadcast_to((NI, 3, 3, NJ)),
                dx3.unsqueeze(1).broadcast_to((NI, 3, 3, NJ)),
                op=ALU.mult))
            # merged reduce: S1 + S2diag (big3[0:864]), then S2off
            vector.wait_ge(gq, GQ_BIGD)
            V(vector.reduce_sum(
                sg[:, 9:27], _v(big3, 0, [[NJ, 18], [1, NJ]]),
                axis=mybir.AxisListType.X,
            ))
            vector.wait_ge(gq, GQ_BIGO)
            V(vector.reduce_sum(
                sg[:, 27:36], _v(big3, 864, [[NJ, 9], [1, NJ]]),
                axis=mybir.AxisListType.X,
            ))
            assert vn[0] == VQ_ALL, vn[0]

    nc.compile()
    return nc


def host_prep(R):
    """Per-core input arrays: [96, 160] = [RjT replicated | Ri | pad]."""
    R = np.ascontiguousarray(R, np.float32)
    in_maps = []
    for core in range(NIB * NJC):
        ib, jc = divmod(core, NJC)
        rji = np.zeros((NI, 160), np.float32)
        rj = R[jc * NJ:(jc + 1) * NJ, :]              # [48, 3]
        rji[:, 0:144] = rj.T.reshape(1, 144)          # d-major, replicated
        rji[:, 144:147] = R[ib * NI:(ib + 1) * NI, :]
        in_maps.append({"rji": rji})
    return in_maps


def host_combine(partials):
    """partials: list of 8 [96,36] arrays (core order). Returns [192,18]."""
    sums = np.zeros((N, 36), np.float64)
    for core, p in enumerate(partials):
        ib = core // NJC
        sums[ib * NI:(ib + 1) * NI] += p.astype(np.float64)
    sums = sums.astype(np.float32)
    q_r = sums[:, 0:9].copy()
    q_r[:, 0] -= 1.0                                  # remove j==i self term
    s0 = q_r[:, 0:3]                                  # [N,3] n=0..2
    s1 = sums[:, 9:18].reshape(N, 3, 3)               # [N,n,d]
    s2d = sums[:, 18:27].reshape(N, 3, 3)             # [N,n,d] diagonal
    s2o = sums[:, 27:36].reshape(N, 3, 3)             # [N,n,m] off-diagonal
    ang = np.empty((N, 3, 3), np.float32)
    ang[:, :, 0] = s0 * s0
    ang[:, :, 1] = (s1 * s1).sum(-1)
    fro2 = (s2d * s2d).sum(-1) + 2.0 * (s2o * s2o).sum(-1)
    ang[:, :, 2] = 1.5 * fro2 - 0.5 * s0 * s0
    return np.concatenate([q_r, ang.reshape(N, 9)], axis=-1)


def _get_nc():
    if "nc" not in _cached:
        _cached["nc"] = build_nc()
    return _cached["nc"]


def kernel(R, box):
    R = np.asarray(R, np.float32)
    box = np.asarray(box, np.float32)
    assert R.shape == (N, 3)
    assert np.allclose(box, np.eye(3, dtype=np.float32) * BOX_L), (
        "kernel compiled for box = 20*I"
    )
    nc = _get_nc()
    in_maps = host_prep(R)
    res = run_bass_kernel_spmd(nc, in_maps, list(range(NIB * NJC)))
    partials = [res.results[c]["out"] for c in range(NIB * NJC)]
    return host_combine(partials)


# revision 21
# speedup vs baseline: 1.0805x; 1.0805x over previous
"""Trainium2 Bass kernel for the N^3 triplet descriptor (gnn_message_passing).

Strategy: the reference's O(N^3) angular sum factorizes exactly via the
Legendre addition theorem into O(N^2) per-pair vector moments:

  P0 term: (sum_j w_j)^2
  P1 term: |sum_j w_j u_j|^2                  (u = unit displacement)
  P2 term: 1.5*|sum_j w_j u_j u_j^T|_F^2 - 0.5*(sum_j w_j)^2

with w_j = fc(r_ij) * r_ij^n.  Each device accumulates 36 pair moments per
central atom (9 radial powers, 9 S1 components, 9+9 symmetric S2
components); the tiny nonlinear combine runs on host after gathering.

All per-pair weights belong to one family e_k = fc * r^(k-2), k=0..10:
radial moments reduce e_2..e_10; S1 weights are e_1..e_3; S2 weights are
e_0..e_2 — a single tile built with 5 strided DVE ops serves everything.

Sharding: 8 cores = 2 i-blocks (96 rows on partitions) x 4 j-chunks (48
neighbors on the free axis). Cross-j-chunk partials are summed on host.

Implementation: raw Bass (no Tile framework) with per-engine semaphore
chains, everything on the DVE (GpSimd shares SBUF ports with the DVE and
slows it ~4x when run concurrently — measured). The single ACT table
(abs_reciprocal_sqrt_and_small) provides 1/r = 1/sqrt(r^2+eps); fc is a
degree-4 polynomial in r^2 with an exact (r^2 < RC^2) cutoff mask.
"""

import numpy as np

import concourse.bass as bass
import concourse.bacc as bacc
from concourse import mybir
from concourse.bass_utils import run_bass_kernel_spmd

F32 = mybir.dt.float32
ALU = mybir.AluOpType
ACT = mybir.ActivationFunctionType

N = 192
NI = 96          # i rows per core (partition dim)
NJ = 48          # j neighbors per core (free dim)
NIB = 2          # i blocks
NJC = 4          # j chunks
BOX_L = 20.0
RC = 5.0
FC_DEG = 6

# fc(w) = 0.5*(1+cos(pi*sqrt(w)/RC)) as poly in w = r^2, w in [0, RC^2]
_FC_W = np.linspace(0, RC * RC, 20001)
_FC_Y = 0.5 * (1 + np.cos(np.pi * np.sqrt(_FC_W) / RC))
_FC_C = (
    np.polynomial.chebyshev.Chebyshev.fit(_FC_W, _FC_Y, FC_DEG, domain=[0, RC * RC])
    .convert(kind=np.polynomial.Polynomial)
    .coef.astype(np.float64)
)

_cached = {}


def _v(ap, off, dims):
    """Custom free-dim view of an SBUF tile AP: keep partition dim, replace
    free dims, shift offset by `off` elements."""
    return bass.AP(ap.tensor, ap.offset + off, [list(ap.ap[0])] + [list(d) for d in dims])


def build_nc():
    nc = bacc.Bacc(
        "TRN2",
        target_bir_lowering=False,
        debug=False,
        enable_asserts=True,
        num_devices=NIB * NJC,
    )
    rji_d = nc.dram_tensor("rji", [NI, 160], F32, kind="ExternalInput").ap()
    out_d = nc.dram_tensor("out", [NI, 36], F32, kind="ExternalOutput").ap()

    rji = nc.alloc_sbuf_tensor("rji_s", [NI, 160], F32).ap()
    dxr = nc.alloc_sbuf_tensor("dxr", [NI, 144], F32).ap()
    # geo = [dx | sq | poff]; products read sq|poff and dx contiguously
    geo = nc.alloc_sbuf_tensor("geo", [NI, 432], F32).ap()
    # rvp = [rinv | r | r2 | r4]
    rvp = nc.alloc_sbuf_tensor("rvp", [NI, 192], F32).ap()
    m25 = nc.alloc_sbuf_tensor("m25", [NI, NJ], F32).ap()
    yh = nc.alloc_sbuf_tensor("yh", [NI, NJ], F32).ap()
    yh144 = nc.alloc_sbuf_tensor("yh144", [NI, 144], F32).ap()
    # wx blocks k=0..10: fc * r^(k-2)
    wx = nc.alloc_sbuf_tensor("wx", [NI, 11 * NJ], F32).ap()
    big3 = nc.alloc_sbuf_tensor("big3", [NI, 1296], F32).ap()  # T | bigd | bigo
    sg = nc.alloc_sbuf_tensor("sg", [NI, 36], F32).ap()
    scr = nc.alloc_sbuf_tensor("scr", [1, 8], F32).ap()

    dsem = nc.alloc_semaphore("dsem")
    vq = nc.alloc_semaphore("vq")      # DVE instruction counter
    sqm = nc.alloc_semaphore("sqm")    # ACT instruction counter

    dx = geo[:, 0:144]
    sq_t = geo[:, 144:288]
    poff = geo[:, 288:432]
    rinv = rvp[:, 0:NJ]
    r = rvp[:, NJ:2 * NJ]
    r2 = rvp[:, 2 * NJ:3 * NJ]
    r4 = rvp[:, 3 * NJ:4 * NJ]
    fc = wx[:, 2 * NJ:3 * NJ]          # e2 = fc * r^0

    rj3 = rji[:, 0:144].rearrange("p (d j) -> p d j", d=3)
    ri3 = rji[:, 144:147].unsqueeze(-1).broadcast_to((NI, 3, NJ))
    dxr3 = dxr.rearrange("p (d j) -> p d j", d=3)

    c = [float(x) for x in _FC_C]

    # cross-engine wait points (per-engine instruction-counter values)
    VQ_R2 = 8                  # r2 (+eps) ready
    VQ_QR = 20 + FC_DEG        # radial moments in sg
    VQ_ALL = 24 + FC_DEG       # sg complete
    SQ_RINV = 2                # rinv ready

    with nc.Block() as block:

        @block.sync
        def _(sync):
            sync.dma_start(rji, rji_d).then_inc(dsem, 16)
            sync.wait_ge(vq, VQ_QR)
            sync.dma_start(out_d[:, 0:9], sg[:, 0:9]).then_inc(dsem, 16)
            sync.wait_ge(vq, VQ_ALL)
            sync.dma_start(out_d[:, 9:36], sg[:, 9:36]).then_inc(dsem, 16)
            sync.wait_ge(dsem, 48)

        @block.scalar
        def _(scalar):
            sn = [0]

            def S(inst):
                # same-engine ordering chain (TRN2 engines pipeline;
                # RAW hazards need explicit sems — free at runtime)
                if sn[0] > 0:
                    inst._wait_ge(sqm, sn[0])
                inst.then_inc(sqm, 1)
                sn[0] += 1
                return inst

            # dummy activation on a const tile: pulls the single ACT table
            # load (abs_reciprocal_sqrt_and_small) to t=0, overlapped with
            # the input DMA + DVE distance math
            S(scalar.activation(
                scr[0:1, 0:1], nc.const_aps.tensor(1.0, (1, 1)),
                ACT.Abs_reciprocal_sqrt))
            scalar.wait_ge(vq, VQ_R2)
            # rinv = 1/sqrt(r2 + 1e-12); r recovered on DVE as r2 * rinv
            S(scalar.activation(rinv, r2, ACT.Abs_reciprocal_sqrt))
            assert sn[0] == SQ_RINV

        @block.vector
        def _(vector):
            vn = [0]

            def V(inst):
                if vn[0] > 0:
                    inst._wait_ge(vq, vn[0])
                inst.then_inc(vq, 1)
                vn[0] += 1
                return inst

            vector.wait_ge(dsem, 16)
            V(vector.tensor_tensor(dxr3, rj3, ri3, op=ALU.subtract))
            # minimum image (box = BOX_L * I): dx -= L*(dxr>L/2); dx += L*(dxr<-L/2)
            V(vector.tensor_scalar(
                yh144, dxr, BOX_L / 2, BOX_L, op0=ALU.is_gt, op1=ALU.mult))
            V(vector.tensor_tensor(dx, dxr, yh144, op=ALU.subtract))
            V(vector.tensor_scalar(
                yh144, dxr, -BOX_L / 2, BOX_L, op0=ALU.is_lt, op1=ALU.mult))
            V(vector.tensor_tensor(dx, dx, yh144, op=ALU.add))
            V(vector.tensor_tensor(sq_t, dx, dx, op=ALU.mult))
            V(vector.reduce_sum(
                r2, sq_t.rearrange("p (d j) -> p j d", d=3),
                axis=mybir.AxisListType.X,
            ))
            # eps keeps 1/sqrt finite at the self pair (u_ii ends up 0)
            V(vector.tensor_scalar(r2, r2, 1e-12, None, op0=ALU.add))
            assert vn[0] == VQ_R2
            # fc = poly(r2) * (r2 < RC^2), Horner on DVE
            V(vector.tensor_scalar(m25, r2, RC * RC, None, op0=ALU.is_lt))
            V(vector.tensor_scalar(yh, r2, c[FC_DEG], None, op0=ALU.mult))
            for k in range(FC_DEG - 1, 0, -1):
                V(vector.scalar_tensor_tensor(
                    yh, yh, c[k], r2, op0=ALU.add, op1=ALU.mult))
            V(vector.scalar_tensor_tensor(
                fc, yh, c[0], m25, op0=ALU.add, op1=ALU.mult))
            # geometry products (independent of fc)
            V(vector.tensor_tensor(
                poff[:, 0:96], dx[:, 0:96], dx[:, 48:144], op=ALU.mult))
            V(vector.tensor_tensor(
                poff[:, 96:144], dx[:, 0:NJ], dx[:, 96:144], op=ALU.mult))
            V(vector.tensor_tensor(r4, r2, r2, op=ALU.mult))
            # weight family e_k = fc * r^(k-2) via strided block multiplies
            vector.wait_ge(sqm, SQ_RINV)
            V(vector.tensor_tensor(r, r2, rinv, op=ALU.mult))
            # [e1|e3] = fc * [rinv|r]
            V(vector.tensor_tensor(
                _v(wx, NJ, [[2 * NJ, 2], [1, NJ]]),
                _v(wx, 2 * NJ, [[0, 2], [1, NJ]]),
                _v(rvp, 0, [[NJ, 2], [1, NJ]]),
                op=ALU.mult))
            V(vector.tensor_tensor(wx[:, 0:NJ], wx[:, NJ:2 * NJ], rinv, op=ALU.mult))
            # [e4|e5] = [e2|e3] * r2
            V(vector.tensor_tensor(
                _v(wx, 4 * NJ, [[NJ, 2], [1, NJ]]),
                _v(wx, 2 * NJ, [[NJ, 2], [1, NJ]]),
                _v(rvp, 2 * NJ, [[0, 2], [1, NJ]]),
                op=ALU.mult))
            # [e6..e9] = [e2..e5] * r4
            V(vector.tensor_tensor(
                _v(wx, 6 * NJ, [[NJ, 4], [1, NJ]]),
                _v(wx, 2 * NJ, [[NJ, 4], [1, NJ]]),
                _v(rvp, 3 * NJ, [[0, 4], [1, NJ]]),
                op=ALU.mult))
            V(vector.tensor_tensor(
                wx[:, 10 * NJ:11 * NJ], wx[:, 6 * NJ:7 * NJ], r4, op=ALU.mult))
            # radial moments: q_r[k] = sum_j e_{k+2}
            V(vector.reduce_sum(
                sg[:, 0:9], _v(wx, 2 * NJ, [[NJ, 9], [1, NJ]]),
                axis=mybir.AxisListType.X,
            ))
            assert vn[0] == VQ_QR
            # S1 products: T[n,d] = e_{n+1} * dx_d -> big3[0:432]
            V(vector.tensor_tensor(
                _v(big3, 0, [[144, 3], [NJ, 3], [1, NJ]]),
                _v(wx, NJ, [[NJ, 3], [0, 3], [1, NJ]]),
                _v(geo, 0, [[0, 3], [NJ, 3], [1, NJ]]),
                op=ALU.mult))
            # S2 products: diag[n,d] = e_n * sq; off[n,m] = e_n * poff
            # (two insts — TensorTensor ISA encodes at most 3 free dims)
            V(vector.tensor_tensor(
                _v(big3, 432, [[144, 3], [NJ, 3], [1, NJ]]),
                _v(wx, 0, [[NJ, 3], [0, 3], [1, NJ]]),
                _v(geo, 144, [[0, 3], [NJ, 3], [1, NJ]]),
                op=ALU.mult))
            V(vector.tensor_tensor(
                _v(big3, 864, [[144, 3], [NJ, 3], [1, NJ]]),
                _v(wx, 0, [[NJ, 3], [0, 3], [1, NJ]]),
                _v(geo, 288, [[0, 3], [NJ, 3], [1, NJ]]),
                op=ALU.mult))
            # merged angular reduce: S1 + S2diag + S2off
            V(vector.reduce_sum(
                sg[:, 9:36], _v(big3, 0, [[NJ, 27], [1, NJ]]),
                axis=mybir.AxisListType.X,
            ))
            assert vn[0] == VQ_ALL, vn[0]

    nc.compile()
    return nc


def host_prep(R):
    """Per-core input arrays: [96, 160] = [RjT replicated | Ri | pad]."""
    R = np.ascontiguousarray(R, np.float32)
    in_maps = []
    for core in range(NIB * NJC):
        ib, jc = divmod(core, NJC)
        rji = np.zeros((NI, 160), np.float32)
        rj = R[jc * NJ:(jc + 1) * NJ, :]              # [48, 3]
        rji[:, 0:144] = rj.T.reshape(1, 144)          # d-major, replicated
        rji[:, 144:147] = R[ib * NI:(ib + 1) * NI, :]
        in_maps.append({"rji": rji})
    return in_maps


def host_combine(partials):
    """partials: list of 8 [96,36] arrays (core order). Returns [192,18]."""
    sums = np.zeros((N, 36), np.float64)
    for core, p in enumerate(partials):
        ib = core // NJC
        sums[ib * NI:(ib + 1) * NI] += p.astype(np.float64)
    sums = sums.astype(np.float32)
    q_r = sums[:, 0:9].copy()
    q_r[:, 0] -= 1.0                                  # remove j==i self term
    s0 = q_r[:, 0:3]                                  # [N,3] n=0..2
    s1 = sums[:, 9:18].reshape(N, 3, 3)               # [N,n,d]
    s2d = sums[:, 18:27].reshape(N, 3, 3)             # [N,n,d] diagonal
    s2o = sums[:, 27:36].reshape(N, 3, 3)             # [N,n,m] off-diagonal
    ang = np.empty((N, 3, 3), np.float32)
    ang[:, :, 0] = s0 * s0
    ang[:, :, 1] = (s1 * s1).sum(-1)
    fro2 = (s2d * s2d).sum(-1) + 2.0 * (s2o * s2o).sum(-1)
    ang[:, :, 2] = 1.5 * fro2 - 0.5 * s0 * s0
    return np.concatenate([q_r, ang.reshape(N, 9)], axis=-1)


def _get_nc():
    if "nc" not in _cached:
        _cached["nc"] = build_nc()
    return _cached["nc"]


def kernel(R, box):
    R = np.asarray(R, np.float32)
    box = np.asarray(box, np.float32)
    assert R.shape == (N, 3)
    assert np.allclose(box, np.eye(3, dtype=np.float32) * BOX_L), (
        "kernel compiled for box = 20*I"
    )
    nc = _get_nc()
    in_maps = host_prep(R)
    res = run_bass_kernel_spmd(nc, in_maps, list(range(NIB * NJC)))
    partials = [res.results[c]["out"] for c in range(NIB * NJC)]
    return host_combine(partials)


# revision 27
# speedup vs baseline: 1.0895x; 1.0083x over previous
"""Trainium2 Bass kernel for the N^3 triplet descriptor (gnn_message_passing).

Strategy: the reference's O(N^3) angular sum factorizes exactly via the
Legendre addition theorem into O(N^2) per-pair vector moments:

  P0 term: (sum_j w_j)^2
  P1 term: |sum_j w_j u_j|^2                  (u = unit displacement)
  P2 term: 1.5*|sum_j w_j u_j u_j^T|_F^2 - 0.5*(sum_j w_j)^2

with w_j = fc(r_ij) * r_ij^n.  Each device accumulates 36 pair moments per
central atom (9 radial powers, 9 S1 components, 9+9 symmetric S2
components); the tiny nonlinear combine runs on host after gathering.

All per-pair weights belong to one family e_k = fc * r^(k-2), k=0..10:
radial moments reduce e_2..e_10; S1 weights are e_1..e_3; S2 weights are
e_0..e_2 — a single tile built with 5 strided DVE ops serves everything.

Sharding: 8 cores = 2 i-blocks (96 rows on partitions) x 4 j-chunks (48
neighbors on the free axis). Cross-j-chunk partials are summed on host.

Implementation: raw Bass (no Tile framework) with per-engine semaphore
chains, everything on the DVE (GpSimd shares SBUF ports with the DVE and
slows it ~4x when run concurrently — measured). The single ACT table
(abs_reciprocal_sqrt_and_small) provides 1/r = 1/sqrt(r^2+eps); fc is a
degree-4 polynomial in r^2 with an exact (r^2 < RC^2) cutoff mask.
"""

import numpy as np

import concourse.bass as bass
import concourse.bacc as bacc
from concourse import mybir
from concourse.bass_utils import run_bass_kernel_spmd

F32 = mybir.dt.float32
ALU = mybir.AluOpType
ACT = mybir.ActivationFunctionType

N = 192
NI = 96          # i rows per core (partition dim)
NJ = 48          # j neighbors per core (free dim)
NIB = 2          # i blocks
NJC = 4          # j chunks
BOX_L = 20.0
RC = 5.0
FC_DEG = 6

# fc(w) = 0.5*(1+cos(pi*sqrt(w)/RC)) as poly in w = r^2, w in [0, RC^2]
_FC_W = np.linspace(0, RC * RC, 20001)
_FC_Y = 0.5 * (1 + np.cos(np.pi * np.sqrt(_FC_W) / RC))
_FC_C = (
    np.polynomial.chebyshev.Chebyshev.fit(_FC_W, _FC_Y, FC_DEG, domain=[0, RC * RC])
    .convert(kind=np.polynomial.Polynomial)
    .coef.astype(np.float64)
)

_cached = {}


def _v(ap, off, dims):
    """Custom free-dim view of an SBUF tile AP: keep partition dim, replace
    free dims, shift offset by `off` elements."""
    return bass.AP(ap.tensor, ap.offset + off, [list(ap.ap[0])] + [list(d) for d in dims])


def build_nc():
    nc = bacc.Bacc(
        "TRN2",
        target_bir_lowering=False,
        debug=False,
        enable_asserts=True,
        num_devices=NIB * NJC,
    )
    rji_d = nc.dram_tensor("rji", [NI, 160], F32, kind="ExternalInput").ap()
    out_d = nc.dram_tensor("out", [NI, 36], F32, kind="ExternalOutput").ap()

    rji = nc.alloc_sbuf_tensor("rji_s", [NI, 160], F32).ap()
    dxr = nc.alloc_sbuf_tensor("dxr", [NI, 144], F32).ap()
    # geo = [dx | sq | poff]; products read sq|poff and dx contiguously
    geo = nc.alloc_sbuf_tensor("geo", [NI, 432], F32).ap()
    # rvp = [rinv | r | r2 | r4]
    rvp = nc.alloc_sbuf_tensor("rvp", [NI, 192], F32).ap()
    m25 = nc.alloc_sbuf_tensor("m25", [NI, NJ], F32).ap()
    yh = nc.alloc_sbuf_tensor("yh", [NI, NJ], F32).ap()
    yh144 = nc.alloc_sbuf_tensor("yh144", [NI, 144], F32).ap()
    # wx blocks k=0..10: fc * r^(k-2)
    wx = nc.alloc_sbuf_tensor("wx", [NI, 11 * NJ], F32).ap()
    big3 = nc.alloc_sbuf_tensor("big3", [NI, 1296], F32).ap()  # T | bigd | bigo
    sg = nc.alloc_sbuf_tensor("sg", [NI, 36], F32).ap()
    scr = nc.alloc_sbuf_tensor("scr", [1, 8], F32).ap()

    dsem = nc.alloc_semaphore("dsem")
    vq = nc.alloc_semaphore("vq")      # DVE instruction counter
    sqm = nc.alloc_semaphore("sqm")    # ACT instruction counter
    gq = nc.alloc_semaphore("gq")      # GpSimd instruction counter

    dx = geo[:, 0:144]
    sq_t = geo[:, 144:288]
    poff = geo[:, 288:432]
    rinv = rvp[:, 0:NJ]
    r = rvp[:, NJ:2 * NJ]
    r2 = rvp[:, 2 * NJ:3 * NJ]
    r4 = rvp[:, 3 * NJ:4 * NJ]
    fc = wx[:, 2 * NJ:3 * NJ]          # e2 = fc * r^0

    rj3 = rji[:, 0:144].rearrange("p (d j) -> p d j", d=3)
    ri3 = rji[:, 144:147].unsqueeze(-1).broadcast_to((NI, 3, NJ))
    dxr3 = dxr.rearrange("p (d j) -> p d j", d=3)

    c = [float(x) for x in _FC_C]

    # cross-engine wait points (per-engine instruction-counter values)
    VQ_DX = 5                  # dx ready
    VQ_R2 = 8                  # r2 (+eps) ready
    VQ_E0 = 13 + FC_DEG        # e0..e3 weight blocks ready
    VQ_QR = 17 + FC_DEG        # radial moments in sg
    VQ_ALL = 21 + FC_DEG       # sg complete
    SQ_RINV = 2                # rinv ready
    GQ_GEO = 3                 # poff + r4 ready
    GQ_BIGO = 4                # S2-off products ready

    with nc.Block() as block:

        @block.sync
        def _(sync):
            sync.dma_start(rji, rji_d).then_inc(dsem, 16)
            sync.wait_ge(vq, VQ_QR)
            sync.dma_start(out_d[:, 0:9], sg[:, 0:9]).then_inc(dsem, 16)
            sync.wait_ge(vq, VQ_ALL)
            sync.dma_start(out_d[:, 9:36], sg[:, 9:36]).then_inc(dsem, 16)
            sync.wait_ge(dsem, 48)

        @block.scalar
        def _(scalar):
            sn = [0]

            def S(inst):
                # same-engine ordering chain (TRN2 engines pipeline;
                # RAW hazards need explicit sems — free at runtime)
                if sn[0] > 0:
                    inst._wait_ge(sqm, sn[0])
                inst.then_inc(sqm, 1)
                sn[0] += 1
                return inst

            # dummy activation on a const tile: pulls the single ACT table
            # load (abs_reciprocal_sqrt_and_small) to t=0, overlapped with
            # the input DMA + DVE distance math
            S(scalar.activation(
                scr[0:1, 0:1], nc.const_aps.tensor(1.0, (1, 1)),
                ACT.Abs_reciprocal_sqrt))
            scalar.wait_ge(vq, VQ_R2)
            # rinv = 1/sqrt(r2 + 1e-12); r recovered on DVE as r2 * rinv
            S(scalar.activation(rinv, r2, ACT.Abs_reciprocal_sqrt))
            assert sn[0] == SQ_RINV

        @block.gpsimd
        def _(gpsimd):
            gn = [0]

            def G(inst):
                if gn[0] > 0:
                    inst._wait_ge(gq, gn[0])
                inst.then_inc(gq, 1)
                gn[0] += 1
                return inst

            # off-critical-path geometry on GpSimd; scheduled against DVE
            # phases with contiguous APs (strided-AP DVE phases suffer ~4x
            # from GpSimd SBUF port contention — measured)
            gpsimd.wait_ge(vq, VQ_DX)
            G(gpsimd.tensor_tensor(
                poff[:, 0:96], dx[:, 0:96], dx[:, 48:144], op=ALU.mult))
            G(gpsimd.tensor_tensor(
                poff[:, 96:144], dx[:, 0:NJ], dx[:, 96:144], op=ALU.mult))
            gpsimd.wait_ge(vq, VQ_R2)
            G(gpsimd.tensor_tensor(r4, r2, r2, op=ALU.mult))
            assert gn[0] == GQ_GEO
            # S2-off products: off[n,m] = e_n * poff -> big3[864:1296]
            gpsimd.wait_ge(vq, VQ_E0)
            G(gpsimd.tensor_tensor(
                _v(big3, 864, [[144, 3], [NJ, 3], [1, NJ]]),
                _v(wx, 0, [[NJ, 3], [0, 3], [1, NJ]]),
                _v(geo, 288, [[0, 3], [NJ, 3], [1, NJ]]),
                op=ALU.mult))
            assert gn[0] == GQ_BIGO

        @block.vector
        def _(vector):
            vn = [0]

            def V(inst):
                if vn[0] > 0:
                    inst._wait_ge(vq, vn[0])
                inst.then_inc(vq, 1)
                vn[0] += 1
                return inst

            vector.wait_ge(dsem, 16)
            V(vector.tensor_tensor(dxr3, rj3, ri3, op=ALU.subtract))
            # minimum image (box = BOX_L * I): dx -= L*(dxr>L/2); dx += L*(dxr<-L/2)
            V(vector.tensor_scalar(
                yh144, dxr, BOX_L / 2, BOX_L, op0=ALU.is_gt, op1=ALU.mult))
            V(vector.tensor_tensor(dx, dxr, yh144, op=ALU.subtract))
            V(vector.tensor_scalar(
                yh144, dxr, -BOX_L / 2, BOX_L, op0=ALU.is_lt, op1=ALU.mult))
            V(vector.tensor_tensor(dx, dx, yh144, op=ALU.add))
            V(vector.tensor_tensor(sq_t, dx, dx, op=ALU.mult))
            V(vector.reduce_sum(
                r2, sq_t.rearrange("p (d j) -> p j d", d=3),
                axis=mybir.AxisListType.X,
            ))
            # eps keeps 1/sqrt finite at the self pair (u_ii ends up 0)
            V(vector.tensor_scalar(r2, r2, 1e-12, None, op0=ALU.add))
            assert vn[0] == VQ_R2
            # fc = poly(r2) * (r2 < RC^2), Horner on DVE
            V(vector.tensor_scalar(m25, r2, RC * RC, None, op0=ALU.is_lt))
            V(vector.tensor_scalar(yh, r2, c[FC_DEG], None, op0=ALU.mult))
            for k in range(FC_DEG - 1, 0, -1):
                V(vector.scalar_tensor_tensor(
                    yh, yh, c[k], r2, op0=ALU.add, op1=ALU.mult))
            V(vector.scalar_tensor_tensor(
                fc, yh, c[0], m25, op0=ALU.add, op1=ALU.mult))
            # weight family e_k = fc * r^(k-2) via strided block multiplies
            vector.wait_ge(sqm, SQ_RINV)
            V(vector.tensor_tensor(r, r2, rinv, op=ALU.mult))
            # [e1|e3] = fc * [rinv|r]
            V(vector.tensor_tensor(
                _v(wx, NJ, [[2 * NJ, 2], [1, NJ]]),
                _v(wx, 2 * NJ, [[0, 2], [1, NJ]]),
                _v(rvp, 0, [[NJ, 2], [1, NJ]]),
                op=ALU.mult))
            V(vector.tensor_tensor(wx[:, 0:NJ], wx[:, NJ:2 * NJ], rinv, op=ALU.mult))
            assert vn[0] == VQ_E0
            # [e4|e5] = [e2|e3] * r2
            V(vector.tensor_tensor(
                _v(wx, 4 * NJ, [[NJ, 2], [1, NJ]]),
                _v(wx, 2 * NJ, [[NJ, 2], [1, NJ]]),
                _v(rvp, 2 * NJ, [[0, 2], [1, NJ]]),
                op=ALU.mult))
            # [e6..e9] = [e2..e5] * r4  (r4 from GpSimd)
            vector.wait_ge(gq, GQ_GEO)
            V(vector.tensor_tensor(
                _v(wx, 6 * NJ, [[NJ, 4], [1, NJ]]),
                _v(wx, 2 * NJ, [[NJ, 4], [1, NJ]]),
                _v(rvp, 3 * NJ, [[0, 4], [1, NJ]]),
                op=ALU.mult))
            V(vector.tensor_tensor(
                wx[:, 10 * NJ:11 * NJ], wx[:, 6 * NJ:7 * NJ], r4, op=ALU.mult))
            # radial moments: q_r[k] = sum_j e_{k+2}
            V(vector.reduce_sum(
                sg[:, 0:9], _v(wx, 2 * NJ, [[NJ, 9], [1, NJ]]),
                axis=mybir.AxisListType.X,
            ))
            assert vn[0] == VQ_QR
            # S1 products: T[n,d] = e_{n+1} * dx_d -> big3[0:432]
            V(vector.tensor_tensor(
                _v(big3, 0, [[144, 3], [NJ, 3], [1, NJ]]),
                _v(wx, NJ, [[NJ, 3], [0, 3], [1, NJ]]),
                _v(geo, 0, [[0, 3], [NJ, 3], [1, NJ]]),
                op=ALU.mult))
            # S2 diag products: diag[n,d] = e_n * sq -> big3[432:864]
            V(vector.tensor_tensor(
                _v(big3, 432, [[144, 3], [NJ, 3], [1, NJ]]),
                _v(wx, 0, [[NJ, 3], [0, 3], [1, NJ]]),
                _v(geo, 144, [[0, 3], [NJ, 3], [1, NJ]]),
                op=ALU.mult))
            # merged reduce S1 + S2diag; S2off reduce last (small final
            # inst: the DVE tail pipe-DRAIN costs ~its duration again)
            V(vector.reduce_sum(
                sg[:, 9:27], _v(big3, 0, [[NJ, 18], [1, NJ]]),
                axis=mybir.AxisListType.X,
            ))
            vector.wait_ge(gq, GQ_BIGO)
            V(vector.reduce_sum(
                sg[:, 27:36], _v(big3, 864, [[NJ, 9], [1, NJ]]),
                axis=mybir.AxisListType.X,
            ))
            assert vn[0] == VQ_ALL, vn[0]

    nc.compile()
    return nc


def host_prep(R):
    """Per-core input arrays: [96, 160] = [RjT replicated | Ri | pad]."""
    R = np.ascontiguousarray(R, np.float32)
    in_maps = []
    for core in range(NIB * NJC):
        ib, jc = divmod(core, NJC)
        rji = np.zeros((NI, 160), np.float32)
        rj = R[jc * NJ:(jc + 1) * NJ, :]              # [48, 3]
        rji[:, 0:144] = rj.T.reshape(1, 144)          # d-major, replicated
        rji[:, 144:147] = R[ib * NI:(ib + 1) * NI, :]
        in_maps.append({"rji": rji})
    return in_maps


def host_combine(partials):
    """partials: list of 8 [96,36] arrays (core order). Returns [192,18]."""
    sums = np.zeros((N, 36), np.float64)
    for core, p in enumerate(partials):
        ib = core // NJC
        sums[ib * NI:(ib + 1) * NI] += p.astype(np.float64)
    sums = sums.astype(np.float32)
    q_r = sums[:, 0:9].copy()
    q_r[:, 0] -= 1.0                                  # remove j==i self term
    s0 = q_r[:, 0:3]                                  # [N,3] n=0..2
    s1 = sums[:, 9:18].reshape(N, 3, 3)               # [N,n,d]
    s2d = sums[:, 18:27].reshape(N, 3, 3)             # [N,n,d] diagonal
    s2o = sums[:, 27:36].reshape(N, 3, 3)             # [N,n,m] off-diagonal
    ang = np.empty((N, 3, 3), np.float32)
    ang[:, :, 0] = s0 * s0
    ang[:, :, 1] = (s1 * s1).sum(-1)
    fro2 = (s2d * s2d).sum(-1) + 2.0 * (s2o * s2o).sum(-1)
    ang[:, :, 2] = 1.5 * fro2 - 0.5 * s0 * s0
    return np.concatenate([q_r, ang.reshape(N, 9)], axis=-1)


def _get_nc():
    if "nc" not in _cached:
        _cached["nc"] = build_nc()
    return _cached["nc"]


def kernel(R, box):
    R = np.asarray(R, np.float32)
    box = np.asarray(box, np.float32)
    assert R.shape == (N, 3)
    assert np.allclose(box, np.eye(3, dtype=np.float32) * BOX_L), (
        "kernel compiled for box = 20*I"
    )
    nc = _get_nc()
    in_maps = host_prep(R)
    res = run_bass_kernel_spmd(nc, in_maps, list(range(NIB * NJC)))
    partials = [res.results[c]["out"] for c in range(NIB * NJC)]
    return host_combine(partials)


# revision 36
# speedup vs baseline: 1.1318x; 1.0388x over previous
"""Trainium2 Bass kernel for the N^3 triplet descriptor (gnn_message_passing).

Strategy: the reference's O(N^3) angular sum factorizes exactly via the
Legendre addition theorem into O(N^2) per-pair vector moments:

  P0 term: (sum_j w_j)^2
  P1 term: |sum_j w_j u_j|^2                  (u = unit displacement)
  P2 term: 1.5*|sum_j w_j u_j u_j^T|_F^2 - 0.5*(sum_j w_j)^2

with w_j = fc(r_ij) * r_ij^n.  Each device accumulates 36 pair moments per
central atom (9 radial powers, 9 S1 components, 9+9 symmetric S2
components); the tiny nonlinear combine runs on host after gathering.

All per-pair weights belong to one family e_k = fc * r^(k-2), k=0..10:
radial moments reduce e_2..e_10; S1 weights are e_1..e_3; S2 weights are
e_0..e_2 — a single tile built with 5 strided DVE ops serves everything.

Sharding: 8 cores = 2 i-blocks (96 rows on partitions) x 4 j-chunks (48
neighbors on the free axis). Cross-j-chunk partials are summed on host.

Implementation: raw Bass (no Tile framework) with per-engine semaphore
chains, everything on the DVE (GpSimd shares SBUF ports with the DVE and
slows it ~4x when run concurrently — measured). The single ACT table
(abs_reciprocal_sqrt_and_small) provides 1/r = 1/sqrt(r^2+eps); fc is a
degree-4 polynomial in r^2 with an exact (r^2 < RC^2) cutoff mask.
"""

import numpy as np

import concourse.bass as bass
import concourse.bacc as bacc
from concourse import mybir
from concourse.bass_utils import run_bass_kernel_spmd

F32 = mybir.dt.float32
ALU = mybir.AluOpType
ACT = mybir.ActivationFunctionType

N = 192
NI = 96          # i rows per core (partition dim)
NJ = 48          # j neighbors per core (free dim)
NIB = 2          # i blocks
NJC = 4          # j chunks
BOX_L = 20.0
RC = 5.0
FC_DEG = 4
R2_EPS = 1e-12

# fc(w) = 0.5*(1+cos(pi*sqrt(w)/RC)) as poly in w = r^2, w in [0, RC^2]
_FC_W = np.linspace(0, RC * RC, 20001)
_FC_Y = 0.5 * (1 + np.cos(np.pi * np.sqrt(_FC_W) / RC))
_FC_C = (
    np.polynomial.chebyshev.Chebyshev.fit(_FC_W, _FC_Y, FC_DEG, domain=[0, RC * RC])
    .convert(kind=np.polynomial.Polynomial)
    .coef.astype(np.float64)
)

_cached = {}


def _v(ap, off, dims):
    """Custom free-dim view of an SBUF tile AP: keep partition dim, replace
    free dims, shift offset by `off` elements."""
    return bass.AP(ap.tensor, ap.offset + off, [list(ap.ap[0])] + [list(d) for d in dims])


def build_nc():
    nc = bacc.Bacc(
        "TRN2",
        target_bir_lowering=False,
        debug=False,
        enable_asserts=True,
        num_devices=NIB * NJC,
    )
    rji_d = nc.dram_tensor("rji", [NI, 160], F32, kind="ExternalInput").ap()
    out_d = nc.dram_tensor("out", [NI, 36], F32, kind="ExternalOutput").ap()

    rji = nc.alloc_sbuf_tensor("rji_s", [NI, 160], F32).ap()
    dxr = nc.alloc_sbuf_tensor("dxr", [NI, 144], F32).ap()
    # geo = [dx | sq | poff]; products read sq|poff and dx contiguously
    geo = nc.alloc_sbuf_tensor("geo", [NI, 432], F32).ap()
    # rvp = [rinv | r | r2 | r4]
    rvp = nc.alloc_sbuf_tensor("rvp", [NI, 192], F32).ap()
    m25 = nc.alloc_sbuf_tensor("m25", [NI, NJ], F32).ap()
    yh = nc.alloc_sbuf_tensor("yh", [NI, NJ], F32).ap()
    yh144 = nc.alloc_sbuf_tensor("yh144", [NI, 144], F32).ap()
    # wx blocks k=0..10: fc * r^(k-2)
    wx = nc.alloc_sbuf_tensor("wx", [NI, 11 * NJ], F32).ap()
    big3 = nc.alloc_sbuf_tensor("big3", [NI, 1296], F32).ap()  # T | bigd | bigo
    sg = nc.alloc_sbuf_tensor("sg", [NI, 36], F32).ap()
    scr = nc.alloc_sbuf_tensor("scr", [1, 8], F32).ap()
    # const for the ACT bias (set by GpSimd at program start)
    c_eps = nc.alloc_sbuf_tensor("c_eps", [128, 1], F32).ap()
    nc.const_aps.aps[(F32, R2_EPS)] = c_eps

    dsem = nc.alloc_semaphore("dsem")
    vq = nc.alloc_semaphore("vq")      # DVE instruction counter
    sqm = nc.alloc_semaphore("sqm")    # ACT instruction counter
    gq = nc.alloc_semaphore("gq")      # GpSimd instruction counter

    dx = geo[:, 0:144]
    sq_t = geo[:, 144:288]
    poff = geo[:, 288:432]
    rinv = rvp[:, 0:NJ]
    r = rvp[:, NJ:2 * NJ]
    r2 = rvp[:, 2 * NJ:3 * NJ]
    r4 = rvp[:, 3 * NJ:4 * NJ]
    fc = wx[:, 2 * NJ:3 * NJ]          # e2 = fc * r^0

    rj3 = rji[:, 0:144].rearrange("p (d j) -> p d j", d=3)
    ri3 = rji[:, 144:147].unsqueeze(-1).broadcast_to((NI, 3, NJ))
    dxr3 = dxr.rearrange("p (d j) -> p d j", d=3)

    c = [float(x) for x in _FC_C]

    # cross-engine wait points (per-engine instruction-counter values)
    VQ_DX = 5                  # dx ready
    VQ_R2 = 7                  # r2 ready
    VQ_QR = 16 + FC_DEG        # radial moments in sg
    VQ_REDA = 19 + FC_DEG      # S1 + S2diag moments in sg
    VQ_ALL = 20 + FC_DEG       # sg complete
    SQ_RINV = 2                # rinv ready
    GQ_EPS = 1                 # c_eps const set
    GQ_GEO = 4                 # poff + r4 ready
    GQ_BIGO = 5                # S2-off products ready

    with nc.Block() as block:

        @block.sync
        def _(sync):
            sync.dma_start(rji, rji_d).then_inc(dsem, 16)
            sync.wait_ge(vq, VQ_QR)
            sync.dma_start(out_d[:, 0:9], sg[:, 0:9]).then_inc(dsem, 16)
            sync.wait_ge(vq, VQ_REDA)
            sync.dma_start(out_d[:, 9:27], sg[:, 9:27]).then_inc(dsem, 16)
            sync.wait_ge(vq, VQ_ALL)
            sync.dma_start(out_d[:, 27:36], sg[:, 27:36]).then_inc(dsem, 16)
            sync.wait_ge(dsem, 64)

        @block.scalar
        def _(scalar):
            sn = [0]

            def S(inst):
                # same-engine ordering chain (TRN2 engines pipeline;
                # RAW hazards need explicit sems — free at runtime)
                if sn[0] > 0:
                    inst._wait_ge(sqm, sn[0])
                inst.then_inc(sqm, 1)
                sn[0] += 1
                return inst

            # dummy activation on a const tile: pulls the single ACT table
            # load (abs_reciprocal_sqrt_and_small) to t=0, overlapped with
            # the input DMA + DVE distance math
            S(scalar.activation(
                scr[0:1, 0:1], nc.const_aps.tensor(1.0, (1, 1)),
                ACT.Abs_reciprocal_sqrt))
            scalar.wait_ge(gq, GQ_EPS)
            scalar.wait_ge(vq, VQ_R2)
            # rinv = 1/sqrt(r2 + eps); r recovered on DVE as r2 * rinv
            S(scalar.activation(rinv, r2, ACT.Abs_reciprocal_sqrt, bias=R2_EPS))
            assert sn[0] == SQ_RINV

        @block.gpsimd
        def _(gpsimd):
            gn = [0]

            def G(inst):
                if gn[0] > 0:
                    inst._wait_ge(gq, gn[0])
                inst.then_inc(gq, 1)
                gn[0] += 1
                return inst

            G(gpsimd.memset(c_eps, R2_EPS))
            # off-critical-path geometry on GpSimd; scheduled against DVE
            # phases with contiguous APs (strided-AP DVE phases suffer ~4x
            # from GpSimd SBUF port contention — measured)
            gpsimd.wait_ge(vq, VQ_DX)
            G(gpsimd.tensor_tensor(
                poff[:, 0:96], dx[:, 0:96], dx[:, 48:144], op=ALU.mult))
            G(gpsimd.tensor_tensor(
                poff[:, 96:144], dx[:, 0:NJ], dx[:, 96:144], op=ALU.mult))
            gpsimd.wait_ge(vq, VQ_R2)
            G(gpsimd.tensor_tensor(r4, r2, r2, op=ALU.mult))
            assert gn[0] == GQ_GEO
            # S2-off products: off[n,m] = e_n * poff -> big3[864:1296].
            # Gated on VQ_QR so it overlaps the DVE's contiguous product
            # phase, not its strided weight-build phase.
            gpsimd.wait_ge(vq, VQ_QR)
            G(gpsimd.tensor_tensor(
                _v(big3, 864, [[144, 3], [NJ, 3], [1, NJ]]),
                _v(wx, 0, [[NJ, 3], [0, 3], [1, NJ]]),
                _v(geo, 288, [[0, 3], [NJ, 3], [1, NJ]]),
                op=ALU.mult))
            assert gn[0] == GQ_BIGO

        @block.vector
        def _(vector):
            vn = [0]

            def V(inst):
                if vn[0] > 0:
                    inst._wait_ge(vq, vn[0])
                inst.then_inc(vq, 1)
                vn[0] += 1
                return inst

            vector.wait_ge(dsem, 16)
            V(vector.tensor_tensor(dxr3, rj3, ri3, op=ALU.subtract))
            # minimum image (box = BOX_L * I): dx -= L*(dxr>L/2); dx += L*(dxr<-L/2)
            V(vector.tensor_scalar(
                yh144, dxr, BOX_L / 2, BOX_L, op0=ALU.is_gt, op1=ALU.mult))
            V(vector.tensor_tensor(dx, dxr, yh144, op=ALU.subtract))
            V(vector.tensor_scalar(
                yh144, dxr, -BOX_L / 2, BOX_L, op0=ALU.is_lt, op1=ALU.mult))
            V(vector.tensor_tensor(dx, dx, yh144, op=ALU.add))
            V(vector.tensor_tensor(sq_t, dx, dx, op=ALU.mult))
            V(vector.reduce_sum(
                r2, sq_t.rearrange("p (d j) -> p j d", d=3),
                axis=mybir.AxisListType.X,
            ))
            assert vn[0] == VQ_R2
            # fc = poly(r2) * (r2 < RC^2), Horner on DVE
            V(vector.tensor_scalar(m25, r2, RC * RC, None, op0=ALU.is_lt))
            V(vector.tensor_scalar(yh, r2, c[FC_DEG], None, op0=ALU.mult))
            for k in range(FC_DEG - 1, 0, -1):
                V(vector.scalar_tensor_tensor(
                    yh, yh, c[k], r2, op0=ALU.add, op1=ALU.mult))
            V(vector.scalar_tensor_tensor(
                fc, yh, c[0], m25, op0=ALU.add, op1=ALU.mult))
            # weight family e_k = fc * r^(k-2) via strided block multiplies
            vector.wait_ge(sqm, SQ_RINV)
            V(vector.tensor_tensor(r, r2, rinv, op=ALU.mult))
            # [e1|e3] = fc * [rinv|r]
            V(vector.tensor_tensor(
                _v(wx, NJ, [[2 * NJ, 2], [1, NJ]]),
                _v(wx, 2 * NJ, [[0, 2], [1, NJ]]),
                _v(rvp, 0, [[NJ, 2], [1, NJ]]),
                op=ALU.mult))
            V(vector.tensor_tensor(wx[:, 0:NJ], wx[:, NJ:2 * NJ], rinv, op=ALU.mult))
            # [e4|e5] = [e2|e3] * r2
            V(vector.tensor_tensor(
                _v(wx, 4 * NJ, [[NJ, 2], [1, NJ]]),
                _v(wx, 2 * NJ, [[NJ, 2], [1, NJ]]),
                _v(rvp, 2 * NJ, [[0, 2], [1, NJ]]),
                op=ALU.mult))
            # [e6..e9] = [e2..e5] * r4  (r4 from GpSimd)
            vector.wait_ge(gq, GQ_GEO)
            V(vector.tensor_tensor(
                _v(wx, 6 * NJ, [[NJ, 4], [1, NJ]]),
                _v(wx, 2 * NJ, [[NJ, 4], [1, NJ]]),
                _v(rvp, 3 * NJ, [[0, 4], [1, NJ]]),
                op=ALU.mult))
            V(vector.tensor_tensor(
                wx[:, 10 * NJ:11 * NJ], wx[:, 6 * NJ:7 * NJ], r4, op=ALU.mult))
            # radial moments: q_r[k] = sum_j e_{k+2}
            V(vector.reduce_sum(
                sg[:, 0:9], _v(wx, 2 * NJ, [[NJ, 9], [1, NJ]]),
                axis=mybir.AxisListType.X,
            ))
            assert vn[0] == VQ_QR
            # S1 products: T[n,d] = e_{n+1} * dx_d -> big3[0:432]
            V(vector.tensor_tensor(
                _v(big3, 0, [[144, 3], [NJ, 3], [1, NJ]]),
                _v(wx, NJ, [[NJ, 3], [0, 3], [1, NJ]]),
                _v(geo, 0, [[0, 3], [NJ, 3], [1, NJ]]),
                op=ALU.mult))
            # S2 diag products: diag[n,d] = e_n * sq -> big3[432:864]
            V(vector.tensor_tensor(
                _v(big3, 432, [[144, 3], [NJ, 3], [1, NJ]]),
                _v(wx, 0, [[NJ, 3], [0, 3], [1, NJ]]),
                _v(geo, 144, [[0, 3], [NJ, 3], [1, NJ]]),
                op=ALU.mult))
            # merged reduce S1 + S2diag; S2off reduce last (small final
            # inst: the DVE tail pipe-DRAIN costs ~its duration again)
            V(vector.reduce_sum(
                sg[:, 9:27], _v(big3, 0, [[NJ, 18], [1, NJ]]),
                axis=mybir.AxisListType.X,
            ))
            assert vn[0] == VQ_REDA
            vector.wait_ge(gq, GQ_BIGO)
            V(vector.reduce_sum(
                sg[:, 27:36], _v(big3, 864, [[NJ, 9], [1, NJ]]),
                axis=mybir.AxisListType.X,
            ))
            assert vn[0] == VQ_ALL, vn[0]

    nc.compile()
    return nc


def host_prep(R):
    """Per-core input arrays: [96, 160] = [RjT replicated | Ri | pad]."""
    R = np.ascontiguousarray(R, np.float32)
    in_maps = []
    for core in range(NIB * NJC):
        ib, jc = divmod(core, NJC)
        rji = np.zeros((NI, 160), np.float32)
        rj = R[jc * NJ:(jc + 1) * NJ, :]              # [48, 3]
        rji[:, 0:144] = rj.T.reshape(1, 144)          # d-major, replicated
        rji[:, 144:147] = R[ib * NI:(ib + 1) * NI, :]
        in_maps.append({"rji": rji})
    return in_maps


def host_combine(partials):
    """partials: list of 8 [96,36] arrays (core order). Returns [192,18]."""
    sums = np.zeros((N, 36), np.float64)
    for core, p in enumerate(partials):
        ib = core // NJC
        sums[ib * NI:(ib + 1) * NI] += p.astype(np.float64)
    sums = sums.astype(np.float32)
    q_r = sums[:, 0:9].copy()
    q_r[:, 0] -= 1.0                                  # remove j==i self term
    s0 = q_r[:, 0:3]                                  # [N,3] n=0..2
    s1 = sums[:, 9:18].reshape(N, 3, 3)               # [N,n,d]
    s2d = sums[:, 18:27].reshape(N, 3, 3)             # [N,n,d] diagonal
    s2o = sums[:, 27:36].reshape(N, 3, 3)             # [N,n,m] off-diagonal
    ang = np.empty((N, 3, 3), np.float32)
    ang[:, :, 0] = s0 * s0
    ang[:, :, 1] = (s1 * s1).sum(-1)
    fro2 = (s2d * s2d).sum(-1) + 2.0 * (s2o * s2o).sum(-1)
    ang[:, :, 2] = 1.5 * fro2 - 0.5 * s0 * s0
    return np.concatenate([q_r, ang.reshape(N, 9)], axis=-1)


def _get_nc():
    if "nc" not in _cached:
        _cached["nc"] = build_nc()
    return _cached["nc"]


def kernel(R, box):
    R = np.asarray(R, np.float32)
    box = np.asarray(box, np.float32)
    assert R.shape == (N, 3)
    assert np.allclose(box, np.eye(3, dtype=np.float32) * BOX_L), (
        "kernel compiled for box = 20*I"
    )
    nc = _get_nc()
    in_maps = host_prep(R)
    res = run_bass_kernel_spmd(nc, in_maps, list(range(NIB * NJC)))
    partials = [res.results[c]["out"] for c in range(NIB * NJC)]
    return host_combine(partials)
